# revision 5
# baseline (speedup 1.0000x reference)
"""Bass/Trainium2 kernel for nn_DecoupledTextDecoder.

Computation (per the reference):
  An   = A / A.sum(spatial)                                 [nB, nT, H*W]
  C    = einsum('bchw,bthw->tbc', feature, An)              [nT, nB, nC]
  hid  = C @ W.T + b                                        [nT, nB, nC]
  pred = hid @ protos.T                                     [nT, nB, nP]
  cos  = pred / (||hid||_2 + 9e-4)
  outRes = concat([pred*ALPHA, UNK], -1); attnMap = concat([cos, UNK], -1)
  scores = scatter-add over labels (identity map) ; ragged pack by textLength

Strategy: pure data-parallel over batch, 16 batches per NeuronCore, all fp32.
Device computes pred*ALPHA and cos for all nT steps; host does the (index-only)
scatter/concat/ragged packing.

Per-core device pipeline (all matmuls contract over the partition dim):
  - feature chunks are PE-transposed (hw -> partitions), A is PE-transposed
    per batch; attention normalization is folded in as a per-row scale of the
    pooled C (C rows are t, scale = 1/Asum[t]).
  - pooled C is transposed to CT [c, (b,t)] tiles of 128 rows (4 batches x 32
    steps), hidden is computed as hiddenT [c_out, tb] = W.T-chunks @ CT,
    decode as [tb, p] = hiddenT-chunks (stationary) @ protosT, which lands in
    the natural [b, t, p] DRAM layout.
  - row norms via squares + ones-matmul; epilogue on ACT (x ALPHA) and DVE
    (x 1/(norm+9e-4)).
"""

import numpy as np
from contextlib import ExitStack

import concourse.bass as bass
import concourse.mybir as mybir
import concourse.tile as tile
from concourse import bacc
from concourse.bass_utils import run_bass_kernel_spmd
from concourse.masks import make_identity

F32 = mybir.dt.float32

NB, NC, NH, NWD = 128, 512, 16, 64
HW = NH * NWD            # 1024
NT = 32                  # attention steps
NP = 4000                # prototypes
NPP = 4096               # padded prototype count (8 x 512)
NCORES = 8
BPC = NB // NCORES       # batches per core = 16
GB = 4                   # batches per group (GB*NT = 128 partitions)
NG = BPC // GB           # groups per core = 4
CCH = NC // 128          # channel chunks = 4
HWCH = HW // 128         # spatial chunks = 8
PCH = NPP // 512         # proto chunks = 8


def _emit(nc, tc, ctx):
    feat = nc.dram_tensor("feat", [BPC, NC, HW], F32, kind="ExternalInput").ap()
    attn = nc.dram_tensor("attn", [BPC, NT, HW], F32, kind="ExternalInput").ap()
    wT = nc.dram_tensor("wT", [NC, NC], F32, kind="ExternalInput").ap()
    pT = nc.dram_tensor("protosT", [NC, NPP], F32, kind="ExternalInput").ap()
    bias = nc.dram_tensor("bias", [NC], F32, kind="ExternalInput").ap()
    alpha = nc.dram_tensor("alpha", [1, 1], F32, kind="ExternalInput").ap()
    pred = nc.dram_tensor("pred", [BPC, NT, NP], F32, kind="ExternalOutput").ap()
    cosm = nc.dram_tensor("cosm", [BPC, NT, NP], F32, kind="ExternalOutput").ap()

    pred_f = pred.rearrange("b t p -> (b t) p")
    cosm_f = cosm.rearrange("b t p -> (b t) p")

    singles = ctx.enter_context(tc.tile_pool(name="singles", bufs=1))
    fpool = ctx.enter_context(tc.tile_pool(name="fpool", bufs=2))
    apool = ctx.enter_context(tc.tile_pool(name="apool", bufs=2))
    ftpool = ctx.enter_context(tc.tile_pool(name="ftpool", bufs=3))
    vpool = ctx.enter_context(tc.tile_pool(name="vpool", bufs=4))
    cpool = ctx.enter_context(tc.tile_pool(name="cpool", bufs=2))
    ctgpool = ctx.enter_context(tc.tile_pool(name="ctgpool", bufs=2))
    hpool = ctx.enter_context(tc.tile_pool(name="hpool", bufs=2))
    opool = ctx.enter_context(tc.tile_pool(name="opool", bufs=4))

    # PSUM budget is 8 banks: tp(2) + c(2) + s(2) + hd(2, shared hidden/decode)
    ps_tp = ctx.enter_context(tc.tile_pool(name="ps_tp", bufs=2, space="PSUM"))
    ps_c = ctx.enter_context(tc.tile_pool(name="ps_c", bufs=2, space="PSUM"))
    ps_s = ctx.enter_context(tc.tile_pool(name="ps_s", bufs=2, space="PSUM"))
    ps_hd = ctx.enter_context(tc.tile_pool(name="ps_hd", bufs=2, space="PSUM"))

    # ---- constants / weights (resident) ----
    ident = singles.tile([128, 128], F32)
    make_identity(nc, ident)
    ones = singles.tile([128, 1], F32)
    nc.vector.memset(ones, 1.0)
    alpha_sb = singles.tile([128, 1], F32)
    nc.gpsimd.dma_start(out=alpha_sb, in_=alpha.to_broadcast((128, 1)))
    bias_sb = singles.tile([128, CCH], F32)
    nc.sync.dma_start(out=bias_sb, in_=bias.rearrange("(k p) -> p k", p=128))
    wT_sb = singles.tile([128, CCH, NC], F32)
    nc.sync.dma_start(out=wT_sb, in_=wT.rearrange("(k p) c -> p k c", p=128))
    pT_sb = singles.tile([128, CCH, NPP], F32)
    nc.sync.dma_start(out=pT_sb, in_=pT.rearrange("(k p) c -> p k c", p=128))

    for g in range(NG):
        ctg = ctgpool.tile([128, CCH, 128], F32, tag="ctg")
        for bb in range(GB):
            b = g * GB + bb
            # ---- load + normalize-stats for A ----
            a_t = apool.tile([NT, HW], F32, tag="a")
            nc.sync.dma_start(out=a_t, in_=attn[b])
            rinv = vpool.tile([NT, 1], F32, tag="rinv")
            nc.vector.reduce_sum(rinv, a_t, axis=mybir.AxisListType.X)
            nc.vector.reciprocal(rinv, rinv)
            # ---- transpose A: [32, 1024] -> AT [128, 8, 32] ----
            at_sb = apool.tile([128, HWCH, NT], F32, tag="at")
            at_ps = ps_s.tile([128, HWCH, NT], F32, tag="sp")
            for j in range(HWCH):
                nc.tensor.transpose(
                    at_ps[:, j, :],
                    a_t[:, j * 128 : (j + 1) * 128],
                    ident[:NT, :NT],
                )
            nc.scalar.copy(at_sb, at_ps)
            # ---- load feature, transpose chunks, pool ----
            f_t = fpool.tile([128, CCH, HW], F32, tag="f")
            nc.sync.dma_start(
                out=f_t, in_=feat[b].rearrange("(cc p) hw -> p cc hw", p=128)
            )
            cps = ps_c.tile([NT, NC], F32, tag="cp")
            for hwc in range(HWCH):
                tp = ps_tp.tile([128, NC], F32, tag="tp")
                for cc in range(CCH):
                    nc.tensor.transpose(
                        tp[:, cc * 128 : (cc + 1) * 128],
                        f_t[:, cc, hwc * 128 : (hwc + 1) * 128],
                        ident,
                    )
                ft = ftpool.tile([128, NC], F32, tag="ft")
                if hwc % 2 == 0:
                    nc.vector.tensor_copy(ft, tp)
                else:
                    nc.scalar.copy(ft, tp)
                nc.tensor.matmul(
                    cps, at_sb[:, hwc, :], ft, start=(hwc == 0), stop=(hwc == HWCH - 1)
                )
            # ---- scale pooled C rows by 1/Asum, transpose into CT group tile ----
            c_sb = apool.tile([NT, NC], F32, tag="c")
            nc.vector.tensor_scalar_mul(c_sb, cps, rinv)
            ct_ps = ps_s.tile([128, CCH, NT], F32, tag="sp")
            for cc in range(CCH):
                nc.tensor.transpose(
                    ct_ps[:, cc, :], c_sb[:, cc * 128 : (cc + 1) * 128], ident[:NT, :NT]
                )
            nc.scalar.copy(ctg[:, :, bb * NT : (bb + 1) * NT], ct_ps)

        # ---- hidden: hT [c_out, tb] = sum_ci wT[ci][:, co] ^T-free @ ctg[ci] ----
        hps = ps_hd.tile([128, CCH, 128], F32, tag="hd")
        for co in range(CCH):
            for ci in range(CCH):
                nc.tensor.matmul(
                    hps[:, co, :],
                    wT_sb[:, ci, co * 128 : (co + 1) * 128],
                    ctg[:, ci, :],
                    start=(ci == 0),
                    stop=(ci == CCH - 1),
                )
        hsb = hpool.tile([128, CCH, 128], F32, tag="h")
        sq = hpool.tile([128, CCH, 128], F32, tag="sq")
        for co in range(CCH):
            nc.vector.tensor_scalar_add(
                hsb[:, co, :], hps[:, co, :], bias_sb[:, co : co + 1]
            )
            nc.scalar.square(sq[:, co, :], hsb[:, co, :])
        nps = ps_s.tile([128, 1], F32, tag="sp")
        for co in range(CCH):
            nc.tensor.matmul(
                nps, sq[:, co, :], ones, start=(co == 0), stop=(co == CCH - 1)
            )
        rs = vpool.tile([128, 1], F32, tag="rs")
        nc.scalar.sqrt(rs, nps)
        nc.vector.tensor_scalar_add(rs, rs, 0.0009)
        nc.vector.reciprocal(rs, rs)

        # ---- decode + epilogue ----
        for pc in range(PCH):
            dps = ps_hd.tile([128, 512], F32, tag="hd")
            for co in range(CCH):
                nc.tensor.matmul(
                    dps,
                    hsb[:, co, :],
                    pT_sb[:, co, pc * 512 : (pc + 1) * 512],
                    start=(co == 0),
                    stop=(co == CCH - 1),
                )
            w = min(512, NP - pc * 512)
            o1 = opool.tile([128, 512], F32, tag="o1")
            o2 = opool.tile([128, 512], F32, tag="o2")
            nc.scalar.mul(o1[:, :w], dps[:, :w], alpha_sb)
            nc.vector.tensor_scalar_mul(o2[:, :w], dps[:, :w], rs)
            rows = slice(g * 128, (g + 1) * 128)
            cols = slice(pc * 512, pc * 512 + w)
            nc.sync.dma_start(out=pred_f[rows, cols], in_=o1[:, :w])
            nc.sync.dma_start(out=cosm_f[rows, cols], in_=o2[:, :w])


def build(num_devices=NCORES):
    nc = bacc.Bacc(
        "TRN2", target_bir_lowering=False, debug=False, num_devices=num_devices
    )
    with tile.TileContext(nc) as tc, ExitStack() as ctx:
        _emit(nc, tc, ctx)
    nc.compile()
    return nc


_nc_cache = {}


def _get_nc(n=NCORES):
    if n not in _nc_cache:
        _nc_cache[n] = build(n)
    return _nc_cache[n]


def make_in_maps(feature, A, protos, W, b, ALPHA):
    wTh = np.ascontiguousarray(W.T.astype(np.float32))
    pTh = np.zeros((NC, NPP), np.float32)
    pTh[:, :NP] = protos.T
    bh = np.ascontiguousarray(b.astype(np.float32))
    ah = np.ascontiguousarray(ALPHA.astype(np.float32).reshape(1, 1))
    maps = []
    for i in range(NCORES):
        sl = slice(i * BPC, (i + 1) * BPC)
        maps.append(
            {
                "feat": np.ascontiguousarray(
                    feature[sl].reshape(BPC, NC, HW).astype(np.float32)
                ),
                "attn": np.ascontiguousarray(
                    A[sl].reshape(BPC, NT, HW).astype(np.float32)
                ),
                "wT": wTh,
                "protosT": pTh,
                "bias": bh,
                "alpha": ah,
            }
        )
    return maps


def _postprocess(pred_btp, cosm_btp, UNK_SCR, labels, textLength):
    """pred_btp/cosm_btp: [nB, nT, nP] device outputs (pred already * ALPHA)."""
    tl = np.asarray(textLength)
    steps = int(tl.max())
    steps = min(steps, NT)
    unk = float(np.asarray(UNK_SCR).reshape(-1)[0])

    nPP1 = len(labels)  # nP + 1
    outRes = np.empty((steps, NB, nPP1), np.float32)
    outRes[:, :, :NP] = pred_btp.transpose(1, 0, 2)[:steps]
    outRes[:, :, NP] = unk
    attnMap = np.empty((steps, NB, nPP1), np.float32)
    attnMap[:, :, :NP] = cosm_btp.transpose(1, 0, 2)[:steps]
    attnMap[:, :, NP] = unk

    labels = np.asarray(labels)
    nClass = int(labels.max()) + 1
    if nClass == nPP1 and np.array_equal(labels, np.arange(nPP1)):
        scores = outRes
    else:
        scores = np.zeros((steps, NB, nClass), outRes.dtype)
        np.add.at(scores, (slice(None), slice(None), labels), outRes)

    used = np.minimum(tl, NT).astype(np.int64)
    t_idx = np.concatenate([np.arange(u) for u in used])
    b_idx = np.repeat(np.arange(NB), used)
    output = scores[t_idx, b_idx, :]
    return output, attnMap


def kernel(feature, A, protos, W, b, UNK_SCR, ALPHA, labels, textLength):
    nc = _get_nc()
    in_maps = make_in_maps(feature, A, protos, W, b, ALPHA)
    res = run_bass_kernel_spmd(nc, in_maps, list(range(NCORES)))
    pred = np.concatenate([res.results[i]["pred"] for i in range(NCORES)], axis=0)
    cosm = np.concatenate([res.results[i]["cosm"] for i in range(NCORES)], axis=0)
    return _postprocess(pred, cosm, UNK_SCR, labels, textLength)
